# revision 6
# baseline (speedup 1.0000x reference)
"""Multi-headed attention TRN2 Bass kernel, chunk-pipelined restructure.

Problem: B=2, S=2048, D=1024, H=16 heads (dh=64), fp32 ref, bool mask.
Sharding: data-parallel over B x tensor-parallel over heads (4 heads /
256 features per core); host sums the 4 partials per batch element.

Key change vs the previous kernel: attention no longer waits for the
whole x stream. ctx for qc 0-2 accumulates chunk-by-chunk (PSUM per
chunk, spill-added to SBUF fp32 between chunks), so scores/exp/ctx for
the kt tiles of chunk c run while chunk c+1 is still streaming in. qc 3
is processed entirely after the stream as a tail that overlaps the 4MB
output drain, with the output projection (p4) of qc 0-2 interleaved
into its attention loop.

Everything else (fp16 matmuls into fp32 PSUM, bit-packed mask with
on-chip expansion + multiplicative AND, [v|1|v] denominator packing,
PSUM-broadcast softmax normalize) is carried over.
"""

import math
from contextlib import ExitStack

import numpy as np

import concourse.mybir as mybir
import concourse.tile as tile
from concourse import bacc
from concourse.bass_utils import run_bass_kernel_spmd

B, S, D, H = 2, 2048, 1024, 16
DH = D // H                 # 64
NCORES = 8
GROUPS = NCORES // B        # 4 head-groups per batch element
FPC = D // GROUPS           # 256 features (4 heads) per core
P = 128
SC = 512                    # q/s chunk (free dim of most matmuls)
NQC = S // SC               # 4
NKT = S // P                # 16 k-position tiles
NKTC = NKT // NQC           # 4 kt tiles per x chunk
NDT = D // P                # 8 contraction tiles over D

F32 = mybir.dt.float32
F16 = mybir.dt.float16
I16 = mybir.dt.int16
W16 = 32                    # packed-mask words per (kt, bitplane)

EXP = mybir.ActivationFunctionType.Exp
MULT = mybir.AluOpType.mult
ADD = mybir.AluOpType.add
AND = mybir.AluOpType.bitwise_and
LSR = mybir.AluOpType.logical_shift_right

# dev bisection knob: "full", "dma", "noattn"
VARIANT = "full"


def _emit(ctx: ExitStack, tc: tile.TileContext, xT, wqT, wkT, wvT, woT, keepT, outT):
    nc = tc.nc

    const = ctx.enter_context(tc.tile_pool(name="const", bufs=1))
    sb = ctx.enter_context(tc.tile_pool(name="sb", bufs=1))
    xtp = ctx.enter_context(tc.tile_pool(name="xtp", bufs=4))
    kbp = ctx.enter_context(tc.tile_pool(name="kbp", bufs=1))
    wp = ctx.enter_context(tc.tile_pool(name="wp", bufs=4))
    stg = ctx.enter_context(tc.tile_pool(name="stg", bufs=3))
    ps = ctx.enter_context(tc.tile_pool(name="ps", bufs=1, space="PSUM"))

    # ---- weights ----
    wq_sb = const.tile([P, NDT, FPC], F16)
    nc.sync.dma_start(wq_sb[:], wqT[:])
    wk_sb = const.tile([P, NDT, FPC], F16)
    nc.scalar.dma_start(wk_sb[:], wkT[:])
    wv_sb = const.tile([P, NDT, FPC], F16)
    nc.gpsimd.dma_start(wv_sb[:], wvT[:])
    wo_sb = const.tile([P, FPC // P, D], F16)
    ones_bc = const.tile([P, DH], F16)
    nc.vector.memset(ones_bc[:], 1.0 / DH)

    # ---- persistent activations ----
    q_sb = [sb.tile([P, S], F16, name=f"q_sb{i}") for i in range(2)]
    k_sb = [sb.tile([P, S], F16, name=f"k_sb{i}") for i in range(2)]
    v_sb = [sb.tile([P, 2, 192], F16, name=f"v_sb{i}") for i in range(NKT)]
    ctx_sb = [sb.tile([P, S], F16, name=f"ctx_sb{i}") for i in range(2)]
    # persistent expanded keep mask, all qc (slices filled as chunks land)
    keep_all = sb.tile([P, NQC, NKT, SC], I16, name="keep_all")
    # fp32 running ctx accumulators for qc 0..2 (qc 3 stays in PSUM in the
    # tail): acc[qc][pair] = (accY, accX), each [P, SC] fp32
    acc = [[(sb.tile([P, SC], F32, name=f"accY_{qc}_{pr}"),
             sb.tile([P, SC], F32, name=f"accX_{qc}_{pr}"))
            for pr in range(2)] for qc in range(NQC - 1)]

    # ---- mask bits + x chunks: one explicitly ordered issue plan ----
    # dma_start only enqueues on the issuing engine's ring, but it occupies
    # a slot in that ENGINE's instruction stream: emitted mid-program on the
    # scalar engine it would not enqueue until ACT chews through every exp
    # before it. Issuing everything first saturates all three rings from
    # t=0, and the per-ring order below matches need-order: x halves before
    # the later-needed mask bits, wo (first used ~135us in) split across
    # sync+gpsimd at the very end. Ring bytes: sync 2.125MB, scalar
    # 1.875MB, gpsimd 2.125MB.
    kb_sb = [kbp.tile([P, NKT, W16], I16, name=f"kb_{qc}")
             for qc in range(NQC)]
    xts = [xtp.tile([P, NDT, SC], F16, tag="xt", name=f"xt_{c}")
           for c in range(NQC)]
    h = NDT // 2

    def xdma(eng, c, half):
        sl = slice(half * h, (half + 1) * h)
        eng.dma_start(xts[c][:, sl, :], xT[c, :, sl, :])

    # sync ring
    xdma(nc.sync, 0, 0)
    xdma(nc.sync, 1, 1)
    xdma(nc.sync, 3, 0)
    nc.sync.dma_start(wo_sb[:, 0, :], woT[:, 0, :])
    # scalar ring
    xdma(nc.scalar, 0, 1)
    xdma(nc.scalar, 2, 0)
    xdma(nc.scalar, 3, 1)
    # gpsimd ring
    nc.gpsimd.dma_start(kb_sb[0][:], keepT[0])
    xdma(nc.gpsimd, 1, 0)
    nc.gpsimd.dma_start(kb_sb[1][:], keepT[1])
    xdma(nc.gpsimd, 2, 1)
    nc.gpsimd.dma_start(kb_sb[2][:], keepT[2])
    nc.gpsimd.dma_start(kb_sb[3][:], keepT[3])
    nc.gpsimd.dma_start(wo_sb[:, 1, :], woT[:, 1, :])

    def expand_keep(qc):
        # keep_all[:, qc] [P, NKT, SC] i16 <- bits kb_sb[qc] (all 16 kt at
        # once: DVE is idle early in the stream, and coarse ops amortize
        # the per-instruction overhead 4x vs per-chunk expansion)
        dst = keep_all[:, qc]
        for b in range(16):
            nc.vector.tensor_scalar(
                dst[:, :, b * W16:(b + 1) * W16], kb_sb[qc][:], b, 1, LSR,
                op1=AND)
        nc.vector.tensor_scalar(dst[:], dst[:], -1, None, MULT)

    def emit_proj_qk(sc_i, xt_sc):
        # q/k projections for chunk sc_i (pair tiles)
        for pair in range(2):
            for wi, (w_sb, dst) in enumerate(((wq_sb, q_sb), (wk_sb, k_sb))):
                mm = ps.tile([P, 2, SC], F32, tag="sc", bufs=2,
                             name=f"qk_{sc_i}_{pair}_{wi}")
                for dt in range(NDT):
                    nc.tensor.matmul(
                        mm[:, 0, :],
                        w_sb[:, dt, pair * P:(pair + 1) * P],
                        xt_sc[:, dt, :],
                        start=(dt == 0),
                        stop=(dt == NDT - 1),
                    )
                nc.scalar.copy(dst[pair][:, sc_i * SC:(sc_i + 1) * SC],
                               mm[:, 0, :])

    def emit_proj_v(sc_i, xt_sc):
        for ssub in range(NKTC):
            kt = sc_i * NKTC + ssub
            vm = ps.tile([P, 2, SC], F32, tag="sc", bufs=2, name=f"v_{kt}")
            for dt in range(NDT):
                nc.tensor.matmul(
                    vm[:, 0, 0:FPC],
                    xt_sc[:, dt, ssub * P:(ssub + 1) * P],
                    wv_sb[:, dt, :],
                    start=(dt == 0),
                    stop=(dt == NDT - 1),
                )
            for pr in range(2):
                nc.vector.tensor_copy(v_sb[kt][:, pr, 0:DH],
                                      vm[:, 0, pr * P:pr * P + DH])
                nc.vector.tensor_copy(v_sb[kt][:, pr, 2 * DH:3 * DH],
                                      vm[:, 0, pr * P + DH:(pr + 1) * P])
            nc.vector.memset(v_sb[kt][:, :, DH:2 * DH], 1.0)

    def attn_iter(qc, pair, kt, keep_sl, ctx_y, ctx_x, first, last):
        qsl = slice(qc * SC, (qc + 1) * SC)
        scb = ps.tile([P, 2, SC], F32, tag="sc", bufs=2,
                      name=f"scb_{qc}_{pair}_{kt}")
        ksl = slice(kt * P, (kt + 1) * P)
        nc.tensor.matmul(
            scb[:, 0, :], k_sb[pair][0:DH, ksl], q_sb[pair][0:DH, qsl],
            start=True, stop=True,
        )
        nc.tensor.matmul(
            scb[:, 1, :], k_sb[pair][DH:P, ksl], q_sb[pair][DH:P, qsl],
            start=True, stop=True, tile_position=(64, 0),
        )
        w = wp.tile([P, 2, SC], F16, tag="w", name=f"w_{qc}_{pair}_{kt}")
        nc.scalar.activation(w[:].rearrange("p h q -> p (h q)"),
                             scb[:].rearrange("p h q -> p (h q)"), EXP)
        kb = keep_sl[:, None, :].to_broadcast((P, 2, SC))
        nc.vector.tensor_tensor(w[:].bitcast(I16), w[:].bitcast(I16), kb, AND)
        vt = v_sb[kt]
        nc.tensor.matmul(
            ctx_y[:], vt[:, pair, 0:2 * DH], w[:, 0, :],
            start=first, stop=last,
        )
        nc.tensor.matmul(
            ctx_x[:], vt[:, pair, DH:3 * DH], w[:, 1, :],
            start=first, stop=last,
        )

    def normalize(qc, pair, ctx_y, ctx_x):
        qsl = slice(qc * SC, (qc + 1) * SC)
        recip = stg.tile([P, SC], F16, tag="recip", name=f"recip_{qc}_{pair}")
        with nc.allow_low_precision(reason="softmax denom reciprocal in fp16"):
            nc.vector.reciprocal(recip[0:DH, :], ctx_x[0:DH, :])
            nc.vector.reciprocal(recip[DH:P, :], ctx_y[DH:P, :])
        nc.tensor.matmul(
            ctx_x[0:DH, :], ones_bc[DH:P, 0:DH], recip[DH:P, :],
            start=True, stop=True, tile_position=(64, 0),
        )
        nc.tensor.matmul(
            ctx_y[DH:P, :], ones_bc[0:DH, 0:DH], recip[0:DH, :],
            start=True, stop=True, tile_position=(0, 64),
        )
        rcp2 = stg.tile([P, SC], F16, tag="recip2", name=f"rcp2_{qc}_{pair}")
        nc.scalar.copy(rcp2[0:DH, :], ctx_x[0:DH, :])
        nc.scalar.copy(rcp2[DH:P, :], ctx_y[DH:P, :])
        nc.vector.tensor_tensor(
            ctx_sb[pair][0:DH, qsl], ctx_y[0:DH, :], rcp2[0:DH, :], MULT)
        nc.vector.tensor_tensor(
            ctx_sb[pair][DH:P, qsl], ctx_x[DH:P, :], rcp2[DH:P, :], MULT)

    out_eng = (nc.sync, nc.scalar, nc.gpsimd)

    # p4 (output projection) machinery: units become drainable once their
    # qc has been normalized in both pairs. qc 0..2 normalize during chunk
    # 3's blocks, so their p4 + out-DMA start before the tail.
    p4_seq = [(qc, ft) for qc in range(NQC - 1) for ft in range(NDT)]
    p4_state = {"ready": 0, "next": 0, "ei": 0, "pend": []}
    sts = {}

    def flush_out_dma():
        pqc, ft0, st = p4_state["pend"].pop(0)
        eng = out_eng[p4_state["ei"] % 3]
        eng.dma_start(outT[pqc, :, ft0:ft0 + 2, :], st[:, ft0:ft0 + 2, :])
        p4_state["ei"] += 1

    def emit_p4_ft(pqc, ft, st):
        pqsl = slice(pqc * SC, (pqc + 1) * SC)
        om = ps.tile([P, 2, SC], F32, tag="sc", bufs=2, name=f"o_{pqc}_{ft}")
        for ph in range(FPC // P):
            nc.tensor.matmul(
                om[:, 0, :],
                wo_sb[:, ph, ft * P:(ft + 1) * P],
                ctx_sb[ph][:, pqsl],
                start=(ph == 0),
                stop=(ph == FPC // P - 1),
            )
        nc.vector.tensor_copy(st[:, ft, :], om[:, 0, :])
        if ft % 2 == 1:
            # lag each out-DMA by one staged pair: by issue time the
            # staging copy has long completed, so even a scalar-ring
            # (ACT-issued) DMA carries no wait that could block exps —
            # all three rings stay usable for the 4MB drain
            p4_state["pend"].append((pqc, ft - 1, st))
            if len(p4_state["pend"]) > 1:
                flush_out_dma()

    def drain_p4(n):
        while (n > 0 and p4_state["next"] < len(p4_seq)
               and p4_state["next"] < p4_state["ready"] * NDT):
            pqc, ft = p4_seq[p4_state["next"]]
            p4_state["next"] += 1
            if pqc not in sts:
                sts[pqc] = stg.tile([P, NDT, SC], F16, tag="stage", bufs=2,
                                    name=f"st_{pqc}")
            emit_p4_ft(pqc, ft, sts[pqc])
            n -= 1

    # ================= stream phase: chunks 0..3 =================
    # Triangular schedule: when chunk c lands, attention runs over the NEW
    # (qc, kt) cells whose operands now exist — (qc < c, kt in chunk c)
    # plus (qc == c, kt 0..4c+3). qc 3 is deferred to the tail. Each
    # (c, qc) block accumulates in PSUM, then spill-adds into the SBUF
    # fp32 running accumulator; the last block per qc folds the
    # accumulator back into PSUM and normalizes there.
    started = set()

    def emit_attn_block(c, pair, qc, kt0, kt1):
        ctx_y = ps.tile([P, SC], F32, tag="cy", bufs=2,
                        name=f"ctxY_{c}_{qc}_{pair}_{kt0}")
        ctx_x = ps.tile([P, SC], F32, tag="cx", bufs=2,
                        name=f"ctxX_{c}_{qc}_{pair}_{kt0}")
        for kt in range(kt0, kt1):
            attn_iter(qc, pair, kt, keep_all[:, qc, kt, :],
                      ctx_y, ctx_x, kt == kt0, kt == kt1 - 1)
            if c == NQC - 1 and pair == 1:
                drain_p4(2)
        ay, ax = acc[qc][pair]
        if c == NQC - 1:
            # fold the running sum back into the final-chunk PSUM,
            # then normalize in place
            nc.vector.tensor_tensor(ctx_y[:], ctx_y[:], ay[:], ADD)
            nc.vector.tensor_tensor(ctx_x[:], ctx_x[:], ax[:], ADD)
            normalize(qc, pair, ctx_y, ctx_x)
            if pair == 1:
                p4_state["ready"] += 1
        elif (qc, pair) not in started:
            # first spill is a plain copy: ACT has a PSUM port and
            # slack here; keeps DVE for the adds
            started.add((qc, pair))
            nc.scalar.copy(ay[:], ctx_y[:])
            nc.scalar.copy(ax[:], ctx_x[:])
        else:
            nc.vector.tensor_tensor(ay[:], ctx_y[:], ay[:], ADD)
            nc.vector.tensor_tensor(ax[:], ctx_x[:], ax[:], ADD)

    for c in range(NQC):
        xt_sc = xts[c]
        # expand qc c's full keep row as soon as its bits can be there
        expand_keep(c)
        if VARIANT == "dma":
            continue
        emit_proj_qk(c, xt_sc)
        if VARIANT == "noattn":
            emit_proj_v(c, xt_sc)
            continue
        # phase A: the (qc==c, old kt) prefix needs only q of this chunk
        # plus k/v of EARLIER chunks — running it here keeps ACT/PE fed
        # while the v projections hold the "sc" PSUM slots
        if 0 < c < NQC - 1:
            for pair in range(2):
                emit_attn_block(c, pair, c, 0, c * NKTC)
        emit_proj_v(c, xt_sc)
        # phase B: everything that needs this chunk's k/v
        blocks = [(qc, c * NKTC, (c + 1) * NKTC) for qc in range(min(c, 3))]
        if c < NQC - 1:
            blocks.append((c, c * NKTC, (c + 1) * NKTC))
        # last chunk: qc-outer so each qc finishes both pairs ASAP -- its
        # normalize unlocks p4 + the out-DMA stream that much earlier
        if c == NQC - 1:
            order = [(pair, blk) for blk in blocks for pair in range(2)]
        else:
            order = [(pair, blk) for pair in range(2) for blk in blocks]
        for pair, (qc, kt0, kt1) in order:
            emit_attn_block(c, pair, qc, kt0, kt1)

    if VARIANT in ("dma", "noattn"):
        for qc in range(NQC):
            st = stg.tile([P, NDT, SC], F16, tag="stage", bufs=2,
                          name=f"zst_{qc}")
            nc.vector.memset(st[:], 0.0)
            out_eng[qc % 3].dma_start(outT[qc], st[:])
        return

    # ================= tail: qc 3 attention + remaining p4 =================
    qc = NQC - 1
    for pair in range(2):
        ctx_y = ps.tile([P, SC], F32, tag="cy", bufs=2, name=f"ctxYt_{pair}")
        ctx_x = ps.tile([P, SC], F32, tag="cx", bufs=2, name=f"ctxXt_{pair}")
        for kt in range(NKT):
            attn_iter(qc, pair, kt, keep_all[:, qc, kt, :],
                      ctx_y, ctx_x, kt == 0, kt == NKT - 1)
            if kt % 2 == 1:
                drain_p4(2)
        normalize(qc, pair, ctx_y, ctx_x)
    drain_p4(len(p4_seq))
    p4_seq.extend((qc, ft) for ft in range(NDT))
    p4_state["ready"] += 1
    drain_p4(NDT)
    while p4_state["pend"]:
        flush_out_dma()


def build():
    nc = bacc.Bacc("TRN2", target_bir_lowering=False, debug=False, num_devices=NCORES)
    xT = nc.dram_tensor("xT", [NQC, P, NDT, SC], F16, kind="ExternalInput").ap()
    wqT = nc.dram_tensor("wqT", [P, NDT, FPC], F16, kind="ExternalInput").ap()
    wkT = nc.dram_tensor("wkT", [P, NDT, FPC], F16, kind="ExternalInput").ap()
    wvT = nc.dram_tensor("wvT", [P, NDT, FPC], F16, kind="ExternalInput").ap()
    woT = nc.dram_tensor("woT", [P, FPC // P, D], F16, kind="ExternalInput").ap()
    keepT = nc.dram_tensor("keepT", [NQC, P, NKT, W16], I16, kind="ExternalInput").ap()
    outT = nc.dram_tensor("outT", [NQC, P, NDT, SC], F16, kind="ExternalOutput").ap()
    with tile.TileContext(nc) as tc, ExitStack() as ctx:
        _emit(ctx, tc, xT, wqT, wkT, wvT, woT, keepT, outT)
    nc.compile()
    return nc


def make_in_maps(query, mask, Wq, Wk, Wv, Wo):
    scale = 1.0 / math.sqrt(DH)
    in_maps = []
    for b in range(B):
        xt = query[b].astype(np.float16).T.reshape(NDT, P, NQC, SC)
        xT = np.ascontiguousarray(xt.transpose(2, 1, 0, 3))
        kp = (~mask[b]).T.astype(np.uint16).reshape(NKT, P, NQC, 16, W16)
        bits = (kp << np.arange(16, dtype=np.uint16)[None, None, None, :, None])
        bits = bits.sum(3, dtype=np.uint16)
        keepT = np.ascontiguousarray(
            bits.transpose(2, 1, 0, 3)).view(np.int16)
        for g in range(GROUPS):
            f0 = g * FPC
            def pack_w(wT):  # [D, FPC] -> [P, NDT, FPC]
                return np.ascontiguousarray(
                    wT.reshape(NDT, P, FPC).transpose(1, 0, 2))
            in_maps.append({
                "xT": xT,
                "wqT": pack_w((Wq[f0:f0 + FPC, :] * scale).T.astype(np.float16)),
                "wkT": pack_w(Wk[f0:f0 + FPC, :].T.astype(np.float16)),
                "wvT": pack_w(Wv[f0:f0 + FPC, :].T.astype(np.float16)),
                "woT": np.ascontiguousarray(
                    Wo[:, f0:f0 + FPC].T.astype(np.float16)
                    .reshape(FPC // P, P, D).transpose(1, 0, 2)),
                "keepT": keepT,
            })
    return in_maps


_NC_CACHE = {}


def _get_nc():
    if "nc" not in _NC_CACHE:
        _NC_CACHE["nc"] = build()
    return _NC_CACHE["nc"]


def gather(results, bo):
    out = np.empty((B, S, D), dtype=np.float32)
    for b in range(B):
        acc = results[b * GROUPS]["outT"].astype(np.float32)
        for g in range(1, GROUPS):
            acc = acc + results[b * GROUPS + g]["outT"].astype(np.float32)
        full = acc.transpose(2, 1, 0, 3).reshape(D, S)
        out[b] = full.T + bo.astype(np.float32)
    return out


def kernel(query, mask, Wq, Wk, Wv, Wo, bo, **kwargs):
    nc = _get_nc()
    in_maps = make_in_maps(np.asarray(query), np.asarray(mask), np.asarray(Wq),
                           np.asarray(Wk), np.asarray(Wv), np.asarray(Wo))
    res = run_bass_kernel_spmd(nc, in_maps, list(range(NCORES)))
    return gather(res.results, np.asarray(bo))


# revision 7
# speedup vs baseline: 1.7448x; 1.7448x over previous
"""Multi-headed attention TRN2 Bass kernel, chunk-pipelined restructure.

Problem: B=2, S=2048, D=1024, H=16 heads (dh=64), fp32 ref, bool mask.
Sharding: data-parallel over B x tensor-parallel over heads (4 heads /
256 features per core); host sums the 4 partials per batch element.

Key change vs the previous kernel: attention no longer waits for the
whole x stream. ctx for qc 0-2 accumulates chunk-by-chunk (PSUM per
chunk, spill-added to SBUF fp32 between chunks), so scores/exp/ctx for
the kt tiles of chunk c run while chunk c+1 is still streaming in. qc 3
is processed entirely after the stream as a tail that overlaps the 4MB
output drain, with the output projection (p4) of qc 0-2 interleaved
into its attention loop.

Everything else (fp16 matmuls into fp32 PSUM, bit-packed mask with
on-chip expansion + multiplicative AND, [v|1|v] denominator packing,
PSUM-broadcast softmax normalize) is carried over.
"""

import math
from contextlib import ExitStack

import numpy as np

import concourse.mybir as mybir
import concourse.tile as tile
from concourse import bacc
from concourse.bass_utils import run_bass_kernel_spmd

B, S, D, H = 2, 2048, 1024, 16
DH = D // H                 # 64
NCORES = 8
GROUPS = NCORES // B        # 4 head-groups per batch element
FPC = D // GROUPS           # 256 features (4 heads) per core
P = 128
SC = 512                    # q/s chunk (free dim of most matmuls)
NQC = S // SC               # 4
NKT = S // P                # 16 k-position tiles
NKTC = NKT // NQC           # 4 kt tiles per x chunk
NDT = D // P                # 8 contraction tiles over D

F32 = mybir.dt.float32
F16 = mybir.dt.float16
I16 = mybir.dt.int16
W16 = 32                    # packed-mask words per (kt, bitplane)

EXP = mybir.ActivationFunctionType.Exp
MULT = mybir.AluOpType.mult
ADD = mybir.AluOpType.add
AND = mybir.AluOpType.bitwise_and
LSR = mybir.AluOpType.logical_shift_right

# dev bisection knob: "full", "dma", "noattn"
VARIANT = "full"


def _emit(ctx: ExitStack, tc: tile.TileContext, xT, wqT, wkT, wvT, woT, keepT, outT):
    nc = tc.nc

    const = ctx.enter_context(tc.tile_pool(name="const", bufs=1))
    sb = ctx.enter_context(tc.tile_pool(name="sb", bufs=1))
    xtp = ctx.enter_context(tc.tile_pool(name="xtp", bufs=4))
    kbp = ctx.enter_context(tc.tile_pool(name="kbp", bufs=1))
    wp = ctx.enter_context(tc.tile_pool(name="wp", bufs=4))
    stg = ctx.enter_context(tc.tile_pool(name="stg", bufs=3))
    ps = ctx.enter_context(tc.tile_pool(name="ps", bufs=1, space="PSUM"))

    # ---- weights ----
    wq_sb = const.tile([P, NDT, FPC], F16)
    wk_sb = const.tile([P, NDT, FPC], F16)
    wv_sb = const.tile([P, NDT, FPC], F16)
    wo_sb = const.tile([P, FPC // P, D], F16)
    ones_bc = const.tile([P, DH], F16)
    nc.vector.memset(ones_bc[:], 1.0 / DH)

    # ---- persistent activations ----
    q_sb = [sb.tile([P, S], F16, name=f"q_sb{i}") for i in range(2)]
    k_sb = [sb.tile([P, S], F16, name=f"k_sb{i}") for i in range(2)]
    v_sb = [sb.tile([P, 2, 192], F16, name=f"v_sb{i}") for i in range(NKT)]
    ctx_sb = [sb.tile([P, S], F16, name=f"ctx_sb{i}") for i in range(2)]
    # persistent expanded keep mask, all qc (slices filled as chunks land)
    keep_all = sb.tile([P, NQC, NKT, SC], I16, name="keep_all")
    # fp32 running ctx accumulators for qc 0..2 (qc 3 stays in PSUM in the
    # tail): acc[qc][pair] = (accY, accX), each [P, SC] fp32
    acc = [[(sb.tile([P, SC], F32, name=f"accY_{qc}_{pr}"),
             sb.tile([P, SC], F32, name=f"accX_{qc}_{pr}"))
            for pr in range(2)] for qc in range(NQC - 1)]

    # ---- mask bits + x chunks: one explicitly ordered issue plan ----
    # dma_start only enqueues on the issuing engine's ring, but it occupies
    # a slot in that ENGINE's instruction stream: emitted mid-program on the
    # scalar engine it would not enqueue until ACT chews through every exp
    # before it. Issuing everything first saturates all three rings from
    # t=0, and the per-ring order below matches need-order: x halves before
    # the later-needed mask bits, wo (first used ~135us in) split across
    # sync+gpsimd at the very end. Ring bytes: sync 2.125MB, scalar
    # 1.875MB, gpsimd 2.125MB.
    kb_sb = [kbp.tile([P, NKT, W16], I16, name=f"kb_{qc}")
             for qc in range(NQC)]
    xts = [xtp.tile([P, NDT, SC], F16, tag="xt", name=f"xt_{c}")
           for c in range(NQC)]
    h = NDT // 2

    def xdma(eng, c, half):
        sl = slice(half * h, (half + 1) * h)
        eng.dma_start(xts[c][:, sl, :], xT[c, :, sl, :])

    # Fill-optimized order: the first attention iteration needs
    # wq+wk+x0 (1.75MB) — x0 ships as quarters placed FIRST on each ring
    # with wq/wk each behind a single quarter, so first-scores starts at
    # ~35us instead of ~48us (ring FIFO at ~20GB/s each).
    q4 = NDT // 4

    def xqdma(eng, c, quarter):
        sl = slice(quarter * q4, (quarter + 1) * q4)
        eng.dma_start(xts[c][:, sl, :], xT[c, :, sl, :])

    # sync ring
    xqdma(nc.sync, 0, 0)
    nc.sync.dma_start(wk_sb[:], wkT[:])
    xdma(nc.sync, 1, 1)
    xdma(nc.sync, 3, 0)
    nc.sync.dma_start(wo_sb[:, 0, :], woT[:, 0, :])
    # scalar ring
    xqdma(nc.scalar, 0, 1)
    nc.scalar.dma_start(wq_sb[:], wqT[:])
    xdma(nc.scalar, 2, 0)
    xdma(nc.scalar, 3, 1)
    # gpsimd ring
    xqdma(nc.gpsimd, 0, 2)
    xqdma(nc.gpsimd, 0, 3)
    nc.gpsimd.dma_start(wv_sb[:], wvT[:])
    nc.gpsimd.dma_start(kb_sb[0][:], keepT[0])
    xdma(nc.gpsimd, 1, 0)
    nc.gpsimd.dma_start(kb_sb[1][:], keepT[1])
    xdma(nc.gpsimd, 2, 1)
    nc.gpsimd.dma_start(kb_sb[2][:], keepT[2])
    nc.gpsimd.dma_start(kb_sb[3][:], keepT[3])
    nc.gpsimd.dma_start(wo_sb[:, 1, :], woT[:, 1, :])

    def expand_keep(qc):
        # keep_all[:, qc] [P, NKT, SC] i16 <- bits kb_sb[qc] (all 16 kt at
        # once: DVE is idle early in the stream, and coarse ops amortize
        # the per-instruction overhead 4x vs per-chunk expansion)
        dst = keep_all[:, qc]
        for b in range(16):
            nc.vector.tensor_scalar(
                dst[:, :, b * W16:(b + 1) * W16], kb_sb[qc][:], b, 1, LSR,
                op1=AND)
        nc.vector.tensor_scalar(dst[:], dst[:], -1, None, MULT)

    def emit_proj_qk(sc_i, xt_sc):
        # q/k projections for chunk sc_i (pair tiles)
        for pair in range(2):
            for wi, (w_sb, dst) in enumerate(((wq_sb, q_sb), (wk_sb, k_sb))):
                mm = ps.tile([P, 2, SC], F32, tag="sc", bufs=2,
                             name=f"qk_{sc_i}_{pair}_{wi}")
                for dt in range(NDT):
                    nc.tensor.matmul(
                        mm[:, 0, :],
                        w_sb[:, dt, pair * P:(pair + 1) * P],
                        xt_sc[:, dt, :],
                        start=(dt == 0),
                        stop=(dt == NDT - 1),
                    )
                nc.scalar.copy(dst[pair][:, sc_i * SC:(sc_i + 1) * SC],
                               mm[:, 0, :])

    def emit_proj_v(sc_i, xt_sc):
        for ssub in range(NKTC):
            kt = sc_i * NKTC + ssub
            vm = ps.tile([P, 2, SC], F32, tag="sc", bufs=2, name=f"v_{kt}")
            for dt in range(NDT):
                nc.tensor.matmul(
                    vm[:, 0, 0:FPC],
                    xt_sc[:, dt, ssub * P:(ssub + 1) * P],
                    wv_sb[:, dt, :],
                    start=(dt == 0),
                    stop=(dt == NDT - 1),
                )
            for pr in range(2):
                nc.vector.tensor_copy(v_sb[kt][:, pr, 0:DH],
                                      vm[:, 0, pr * P:pr * P + DH])
                nc.vector.tensor_copy(v_sb[kt][:, pr, 2 * DH:3 * DH],
                                      vm[:, 0, pr * P + DH:(pr + 1) * P])
            nc.vector.memset(v_sb[kt][:, :, DH:2 * DH], 1.0)

    def attn_iter(qc, pair, kt, keep_sl, ctx_y, ctx_x, first, last):
        qsl = slice(qc * SC, (qc + 1) * SC)
        scb = ps.tile([P, 2, SC], F32, tag="sc", bufs=2,
                      name=f"scb_{qc}_{pair}_{kt}")
        ksl = slice(kt * P, (kt + 1) * P)
        nc.tensor.matmul(
            scb[:, 0, :], k_sb[pair][0:DH, ksl], q_sb[pair][0:DH, qsl],
            start=True, stop=True,
        )
        nc.tensor.matmul(
            scb[:, 1, :], k_sb[pair][DH:P, ksl], q_sb[pair][DH:P, qsl],
            start=True, stop=True, tile_position=(64, 0),
        )
        w = wp.tile([P, 2, SC], F16, tag="w", name=f"w_{qc}_{pair}_{kt}")
        nc.scalar.activation(w[:].rearrange("p h q -> p (h q)"),
                             scb[:].rearrange("p h q -> p (h q)"), EXP)
        kb = keep_sl[:, None, :].to_broadcast((P, 2, SC))
        nc.vector.tensor_tensor(w[:].bitcast(I16), w[:].bitcast(I16), kb, AND)
        vt = v_sb[kt]
        nc.tensor.matmul(
            ctx_y[:], vt[:, pair, 0:2 * DH], w[:, 0, :],
            start=first, stop=last,
        )
        nc.tensor.matmul(
            ctx_x[:], vt[:, pair, DH:3 * DH], w[:, 1, :],
            start=first, stop=last,
        )

    def normalize(qc, pair, ctx_y, ctx_x):
        qsl = slice(qc * SC, (qc + 1) * SC)
        recip = stg.tile([P, SC], F16, tag="recip", name=f"recip_{qc}_{pair}")
        with nc.allow_low_precision(reason="softmax denom reciprocal in fp16"):
            nc.vector.reciprocal(recip[0:DH, :], ctx_x[0:DH, :])
            nc.vector.reciprocal(recip[DH:P, :], ctx_y[DH:P, :])
        nc.tensor.matmul(
            ctx_x[0:DH, :], ones_bc[DH:P, 0:DH], recip[DH:P, :],
            start=True, stop=True, tile_position=(64, 0),
        )
        nc.tensor.matmul(
            ctx_y[DH:P, :], ones_bc[0:DH, 0:DH], recip[0:DH, :],
            start=True, stop=True, tile_position=(0, 64),
        )
        rcp2 = stg.tile([P, SC], F16, tag="recip2", name=f"rcp2_{qc}_{pair}")
        nc.scalar.copy(rcp2[0:DH, :], ctx_x[0:DH, :])
        nc.scalar.copy(rcp2[DH:P, :], ctx_y[DH:P, :])
        nc.vector.tensor_tensor(
            ctx_sb[pair][0:DH, qsl], ctx_y[0:DH, :], rcp2[0:DH, :], MULT)
        nc.vector.tensor_tensor(
            ctx_sb[pair][DH:P, qsl], ctx_x[DH:P, :], rcp2[DH:P, :], MULT)

    out_eng = (nc.sync, nc.scalar, nc.gpsimd)

    # p4 (output projection) machinery: units become drainable once their
    # qc has been normalized in both pairs. qc 0..2 normalize during chunk
    # 3's blocks, so their p4 + out-DMA start before the tail.
    p4_seq = [(qc, ft) for qc in range(NQC - 1) for ft in range(NDT)]
    p4_state = {"ready": 0, "next": 0, "ei": 0, "pend": []}
    sts = {}

    def flush_out_dma():
        pqc, ft0, st = p4_state["pend"].pop(0)
        eng = out_eng[p4_state["ei"] % 3]
        eng.dma_start(outT[pqc, :, ft0:ft0 + 2, :], st[:, ft0:ft0 + 2, :])
        p4_state["ei"] += 1

    def emit_p4_ft(pqc, ft, st):
        pqsl = slice(pqc * SC, (pqc + 1) * SC)
        om = ps.tile([P, 2, SC], F32, tag="sc", bufs=2, name=f"o_{pqc}_{ft}")
        for ph in range(FPC // P):
            nc.tensor.matmul(
                om[:, 0, :],
                wo_sb[:, ph, ft * P:(ft + 1) * P],
                ctx_sb[ph][:, pqsl],
                start=(ph == 0),
                stop=(ph == FPC // P - 1),
            )
        nc.vector.tensor_copy(st[:, ft, :], om[:, 0, :])
        if ft % 2 == 1:
            # lag each out-DMA by one staged pair: by issue time the
            # staging copy has long completed, so even a scalar-ring
            # (ACT-issued) DMA carries no wait that could block exps —
            # all three rings stay usable for the 4MB drain
            p4_state["pend"].append((pqc, ft - 1, st))
            if len(p4_state["pend"]) > 1:
                flush_out_dma()

    def drain_p4(n):
        while (n > 0 and p4_state["next"] < len(p4_seq)
               and p4_state["next"] < p4_state["ready"] * NDT):
            pqc, ft = p4_seq[p4_state["next"]]
            p4_state["next"] += 1
            if pqc not in sts:
                sts[pqc] = stg.tile([P, NDT, SC], F16, tag="stage", bufs=2,
                                    name=f"st_{pqc}")
            emit_p4_ft(pqc, ft, sts[pqc])
            n -= 1

    # ================= stream phase: chunks 0..3 =================
    # Triangular schedule: when chunk c lands, attention runs over the NEW
    # (qc, kt) cells whose operands now exist — (qc < c, kt in chunk c)
    # plus (qc == c, kt 0..4c+3). qc 3 is deferred to the tail. Each
    # (c, qc) block accumulates in PSUM, then spill-adds into the SBUF
    # fp32 running accumulator; the last block per qc folds the
    # accumulator back into PSUM and normalizes there.
    started = set()

    def emit_attn_block(c, pair, qc, kt0, kt1):
        ctx_y = ps.tile([P, SC], F32, tag="cy", bufs=2,
                        name=f"ctxY_{c}_{qc}_{pair}_{kt0}")
        ctx_x = ps.tile([P, SC], F32, tag="cx", bufs=2,
                        name=f"ctxX_{c}_{qc}_{pair}_{kt0}")
        for kt in range(kt0, kt1):
            attn_iter(qc, pair, kt, keep_all[:, qc, kt, :],
                      ctx_y, ctx_x, kt == kt0, kt == kt1 - 1)
            if c == NQC - 1 and pair == 1:
                drain_p4(2)
        ay, ax = acc[qc][pair]
        if c == NQC - 1:
            # fold the running sum back into the final-chunk PSUM,
            # then normalize in place
            nc.vector.tensor_tensor(ctx_y[:], ctx_y[:], ay[:], ADD)
            nc.vector.tensor_tensor(ctx_x[:], ctx_x[:], ax[:], ADD)
            normalize(qc, pair, ctx_y, ctx_x)
            if pair == 1:
                p4_state["ready"] += 1
        elif (qc, pair) not in started:
            # first spill is a plain copy: ACT has a PSUM port and
            # slack here; keeps DVE for the adds
            started.add((qc, pair))
            nc.scalar.copy(ay[:], ctx_y[:])
            nc.scalar.copy(ax[:], ctx_x[:])
        else:
            nc.vector.tensor_tensor(ay[:], ctx_y[:], ay[:], ADD)
            nc.vector.tensor_tensor(ax[:], ctx_x[:], ax[:], ADD)

    for c in range(NQC):
        xt_sc = xts[c]
        # expand qc c's full keep row as soon as its bits can be there
        expand_keep(c)
        if VARIANT == "dma":
            continue
        emit_proj_qk(c, xt_sc)
        if VARIANT == "noattn":
            emit_proj_v(c, xt_sc)
            continue
        # phase A: the (qc==c, old kt) prefix needs only q of this chunk
        # plus k/v of EARLIER chunks — running it here keeps ACT/PE fed
        # while the v projections hold the "sc" PSUM slots
        if 0 < c < NQC - 1:
            for pair in range(2):
                emit_attn_block(c, pair, c, 0, c * NKTC)
        emit_proj_v(c, xt_sc)
        # phase B: everything that needs this chunk's k/v
        blocks = [(qc, c * NKTC, (c + 1) * NKTC) for qc in range(min(c, 3))]
        if c < NQC - 1:
            blocks.append((c, c * NKTC, (c + 1) * NKTC))
        # last chunk: qc-outer so each qc finishes both pairs ASAP -- its
        # normalize unlocks p4 + the out-DMA stream that much earlier
        if c == NQC - 1:
            order = [(pair, blk) for blk in blocks for pair in range(2)]
        else:
            order = [(pair, blk) for pair in range(2) for blk in blocks]
        for pair, (qc, kt0, kt1) in order:
            emit_attn_block(c, pair, qc, kt0, kt1)

    if VARIANT in ("dma", "noattn"):
        for qc in range(NQC):
            st = stg.tile([P, NDT, SC], F16, tag="stage", bufs=2,
                          name=f"zst_{qc}")
            nc.vector.memset(st[:], 0.0)
            out_eng[qc % 3].dma_start(outT[qc], st[:])
        return

    # ================= tail: qc 3 attention + remaining p4 =================
    qc = NQC - 1
    for pair in range(2):
        ctx_y = ps.tile([P, SC], F32, tag="cy", bufs=2, name=f"ctxYt_{pair}")
        ctx_x = ps.tile([P, SC], F32, tag="cx", bufs=2, name=f"ctxXt_{pair}")
        for kt in range(NKT):
            attn_iter(qc, pair, kt, keep_all[:, qc, kt, :],
                      ctx_y, ctx_x, kt == 0, kt == NKT - 1)
            if kt % 2 == 1:
                drain_p4(2)
        normalize(qc, pair, ctx_y, ctx_x)
    drain_p4(len(p4_seq))
    p4_seq.extend((qc, ft) for ft in range(NDT))
    p4_state["ready"] += 1
    drain_p4(NDT)
    while p4_state["pend"]:
        flush_out_dma()


def build():
    nc = bacc.Bacc("TRN2", target_bir_lowering=False, debug=False, num_devices=NCORES)
    xT = nc.dram_tensor("xT", [NQC, P, NDT, SC], F16, kind="ExternalInput").ap()
    wqT = nc.dram_tensor("wqT", [P, NDT, FPC], F16, kind="ExternalInput").ap()
    wkT = nc.dram_tensor("wkT", [P, NDT, FPC], F16, kind="ExternalInput").ap()
    wvT = nc.dram_tensor("wvT", [P, NDT, FPC], F16, kind="ExternalInput").ap()
    woT = nc.dram_tensor("woT", [P, FPC // P, D], F16, kind="ExternalInput").ap()
    keepT = nc.dram_tensor("keepT", [NQC, P, NKT, W16], I16, kind="ExternalInput").ap()
    outT = nc.dram_tensor("outT", [NQC, P, NDT, SC], F16, kind="ExternalOutput").ap()
    with tile.TileContext(nc) as tc, ExitStack() as ctx:
        _emit(ctx, tc, xT, wqT, wkT, wvT, woT, keepT, outT)
    nc.compile()
    return nc


def make_in_maps(query, mask, Wq, Wk, Wv, Wo):
    scale = 1.0 / math.sqrt(DH)
    in_maps = []
    for b in range(B):
        xt = query[b].astype(np.float16).T.reshape(NDT, P, NQC, SC)
        xT = np.ascontiguousarray(xt.transpose(2, 1, 0, 3))
        kp = (~mask[b]).T.astype(np.uint16).reshape(NKT, P, NQC, 16, W16)
        bits = (kp << np.arange(16, dtype=np.uint16)[None, None, None, :, None])
        bits = bits.sum(3, dtype=np.uint16)
        keepT = np.ascontiguousarray(
            bits.transpose(2, 1, 0, 3)).view(np.int16)
        for g in range(GROUPS):
            f0 = g * FPC
            def pack_w(wT):  # [D, FPC] -> [P, NDT, FPC]
                return np.ascontiguousarray(
                    wT.reshape(NDT, P, FPC).transpose(1, 0, 2))
            in_maps.append({
                "xT": xT,
                "wqT": pack_w((Wq[f0:f0 + FPC, :] * scale).T.astype(np.float16)),
                "wkT": pack_w(Wk[f0:f0 + FPC, :].T.astype(np.float16)),
                "wvT": pack_w(Wv[f0:f0 + FPC, :].T.astype(np.float16)),
                "woT": np.ascontiguousarray(
                    Wo[:, f0:f0 + FPC].T.astype(np.float16)
                    .reshape(FPC // P, P, D).transpose(1, 0, 2)),
                "keepT": keepT,
            })
    return in_maps


_NC_CACHE = {}


def _get_nc():
    if "nc" not in _NC_CACHE:
        _NC_CACHE["nc"] = build()
    return _NC_CACHE["nc"]


def gather(results, bo):
    out = np.empty((B, S, D), dtype=np.float32)
    for b in range(B):
        acc = results[b * GROUPS]["outT"].astype(np.float32)
        for g in range(1, GROUPS):
            acc = acc + results[b * GROUPS + g]["outT"].astype(np.float32)
        full = acc.transpose(2, 1, 0, 3).reshape(D, S)
        out[b] = full.T + bo.astype(np.float32)
    return out


def kernel(query, mask, Wq, Wk, Wv, Wo, bo, **kwargs):
    nc = _get_nc()
    in_maps = make_in_maps(np.asarray(query), np.asarray(mask), np.asarray(Wq),
                           np.asarray(Wk), np.asarray(Wv), np.asarray(Wo))
    res = run_bass_kernel_spmd(nc, in_maps, list(range(NCORES)))
    return gather(res.results, np.asarray(bo))
